# revision 31
# baseline (speedup 1.0000x reference)
"""BitLinear (RMSNorm + ternary-quantized linear) on 8 trn2 NeuronCores.

Reference math (fp32):
    xn   = x * rsqrt(mean(x^2, -1) + 1e-5) * gamma          # [B,S,K]
    s    = max(mean(|w|), 1e-5)                             # scalar
    q    = round(clip(w / s, -1, 1))                        # ternary {-1,0,1}
    out  = (xn @ q.T) * s                                   # [B,S,Dout]

Identities used by the kernel:
    q = (w > s/2) - (w < -s/2)   (exact, incl. round-half-even at |wn|=0.5)
    out[t,o] = inv[t] * s * sum_k (x[t,k]*gamma[k]) * q[o,k]
so gamma folds into x before the fp16 cast, q is exactly ternary in fp16,
and inv*s folds into the PSUM->SBUF epilogue. The contraction runs on the
PE in fp16 with fp32 PSUM accumulation.

Sharding: 2 token-groups x 4 dout-groups (core = rg*4 + cg).

Scale collective: each core's w_shT IS a full cg-quarter of W, so the
|w|-partials of cores {0..3} (cg 0..3) already cover all of W; same for
{4..7}.  AllReduce over replica_groups [[0,1,2,3],[4,5,6,7]] therefore
yields the exact full-sum on every core while each core only rendezvouses
with 3 peers (halves the launch-skew wait vs an 8-way group), and no extra
w_red slice needs to be read at all.  The AR payload is pre-reduced to a
single scalar (partition-sum on the PE) so the collective moves 4 bytes.

Layout notes:
  - x strips [128 tok, K] are transposed on-chip (xbar DMA transpose of the
    fp16 copy) into xT[p, t, tok] with k = t*128 + p, so the contraction dim
    sits on partitions for the PE.
  - w_shT k-tiles 0..RES_W-1 stay resident in SBUF from the partial pass;
    the rest re-stream during quantize (prefetch hides most of it).
  - quantize is 2 DVE ops/tile: nm=(w<-t); q=(w>t)-nm  (scalar_tensor_tensor).
  - main loop is j-outer / t-mid / d-inner (4 psum banks per strip) so the
    PE consumes qQ k-tiles in the order quantize produces them.
  - prep_strip is s-independent (inv*s computed per block after the AR), so
    all x-prep can fill the collective wait.
"""

import numpy as np

import concourse.bass as bass
import concourse.tile as tile
from concourse import bacc, mybir
from concourse.bass_utils import run_bass_kernel_spmd

F32 = mybir.dt.float32
F16 = mybir.dt.float16

# Full-problem constants
B, S, K, DOUT = 4, 2048, 2048, 8192
N_CORES = 8
RG, CG = 2, 4  # token groups x dout groups
TOK_SH = (B * S) // RG     # 4096 tokens per core
DOUT_SH = DOUT // CG       # 2048 out-features per core
EPS = 1e-5
W_COUNT = float(DOUT * K)  # 16777216
RES_W = 5                  # w k-tiles kept resident in SBUF


def build_nc(tok_sh=TOK_SH, k=K, dout_sh=DOUT_SH,
             w_count=W_COUNT, n_cores=N_CORES, use_cc=True,
             debug_fixed_scale=None, strip_blk=4, res_w=RES_W):
    """Build the SPMD Bass program (one program, per-core inputs differ)."""
    kt = k // 128            # contraction tiles (16)
    n_strip = tok_sh // 128  # token strips (32)
    strip_blk = min(strip_blk, n_strip)
    n_blk = n_strip // strip_blk

    nc = bacc.Bacc("TRN2", target_bir_lowering=False, num_devices=n_cores)

    x_d = nc.declare_dram_parameter("x_sh", [tok_sh, k], F32, isOutput=False)
    w_d = nc.declare_dram_parameter("w_shT", [k, dout_sh], F32, isOutput=False)
    g_d = nc.declare_dram_parameter("gamma", [k], F32, isOutput=False)
    out_d = nc.declare_dram_parameter("out_sh", [tok_sh, dout_sh], F32, isOutput=True)

    with tile.TileContext(nc, num_cores=n_cores) as tc:
        with (
            tc.tile_pool(name="consts", bufs=1) as consts,
            tc.tile_pool(name="wres", bufs=1) as wres,
            tc.tile_pool(name="f32s", bufs=1) as f32s,
            tc.tile_pool(name="f16s", bufs=1) as f16s,
            tc.tile_pool(name="partials", bufs=1) as partials,
            tc.tile_pool(name="qt", bufs=1) as qtp,
            tc.tile_pool(name="outp", bufs=1) as outp,
            tc.tile_pool(name="psum", bufs=8, space="PSUM") as psum,
            tc.tile_pool(name="dram", bufs=1, space="DRAM") as dram,
        ):
            # ---- packed constants block (one tile, disjoint column ranges) -
            # cols: 0 ones_col, 1 eps, 2 prev, 3 tot_sb, 4..19 parts,
            #       20 allv, 21 s_mean, 22 s_clip, 23 s_bc, 24 t_bc, 25 nt_bc,
            #       26..57 inv[strip], 58..89 inv*s[strip],
            #       row0 of 96..223: ones_row
            cblock = consts.tile([128, 224], F32)
            halfn_col = cblock[:, 0:1]
            eps_t = cblock[:, 1:2]
            prev = cblock[:, 2:3]
            tot_sb = cblock[:, 3:4]
            parts = cblock[:, 4:4 + kt]
            allv = cblock[:, 20:21]
            t_bc = cblock[:, 24:25]
            nt_bc = cblock[:, 25:26]
            invblk = cblock[:, 26:26 + n_strip]
            invsblk = cblock[:, 58:58 + n_strip]
            nc.vector.memset(halfn_col, 0.5 / w_count)
            nc.vector.memset(eps_t, EPS)
            gamma_rep = consts.tile([128, k], F32)
            g_bcast = bass.AP(tensor=g_d, offset=0, ap=[[0, 128], [1, k]])
            nc.gpsimd.dma_start(out=gamma_rep, in_=g_bcast)

            # ---- phase W1: |w| partials from own w_shT + group AllReduce ---
            # w loads split across the sync and scalar DMA queues for
            # bandwidth; nothing else competes during this window.
            wtiles = {}
            for i in range(kt):
                if i < res_w:
                    wt = wres.tile([128, dout_sh], F32, tag=f"w{i}",
                                   name=f"w{i}")
                    wtiles[i] = wt
                else:
                    wt = f32s.tile([128, dout_sh], F32, tag="wstage", bufs=3,
                                   name=f"wp{i}")
                weng = nc.sync if i % 2 == 0 else nc.scalar
                weng.dma_start(out=wt, in_=w_d[i * 128:(i + 1) * 128, :])
                nc.vector.tensor_reduce(
                    parts[:, i:i + 1], wt, axis=mybir.AxisListType.X,
                    op=mybir.AluOpType.add, apply_absolute_value=True)

            if debug_fixed_scale is not None:
                nc.vector.memset(t_bc, debug_fixed_scale * 0.5)
                nc.vector.memset(nt_bc, -debug_fixed_scale * 0.5)
            else:
                nc.vector.tensor_reduce(prev, parts, axis=mybir.AxisListType.X,
                                        op=mybir.AluOpType.add)
                # Partition-sum against a 0.5/count column: the AR payload IS
                # the threshold numerator (t = max(sum*0.5/count, eps/2)), so
                # nothing post-AR needs the PE or ACT.  Pre-AR hops hide
                # inside the ~30us CC pickup latency; post-AR is one
                # broadcast DMA + two DVE ops feeding the DVE quantize —
                # all on a single queue, no cross-engine latency.
                tot_ps = psum.tile([1, 1], F32, tag="mm")
                nc.tensor.matmul(tot_ps, lhsT=prev, rhs=halfn_col,
                                 start=True, stop=True)
                nc.vector.tensor_copy(tot_sb[0:1, :], tot_ps)

                cc_in = dram.tile([1, 1], F32)
                cc_out = dram.tile([1, 1], F32)
                nc.gpsimd.dma_start(out=cc_in, in_=tot_sb[0:1, :])
                if use_cc:
                    # each 4-group covers all 4 cg quarters of W -> exact sum
                    nc.gpsimd.collective_compute(
                        "AllReduce", mybir.AluOpType.add,
                        replica_groups=[[0, 1, 2, 3], [4, 5, 6, 7]],
                        ins=[cc_in.opt()], outs=[cc_out.opt()],
                    )
                else:
                    nc.sync.dma_start(out=cc_out, in_=cc_in)
                # broadcast the 4B result across partitions in the DMA itself
                cc_ap = cc_out[0:1, 0:1]
                cc_bcast = bass.AP(tensor=cc_ap.tensor, offset=cc_ap.offset,
                                   ap=[[0, 128], [1, 1]])
                nc.gpsimd.dma_start(out=allv, in_=cc_bcast)
                nc.vector.tensor_scalar_max(t_bc, allv, EPS * 0.5)
                nc.vector.tensor_scalar_mul(nt_bc, t_bc, -1.0)

            # ---- per-strip prep (fully s-independent) ----------------------
            xT_tiles = {}     # strip j -> xT tile (rotating pool slots)

            def prep_strip(j):
                xf = f32s.tile([128, k], F32, tag="big32", bufs=2,
                               name=f"xf{j}")
                nc.gpsimd.dma_start(out=xf, in_=x_d[j * 128:(j + 1) * 128, :])
                xsq = f32s.tile([128, k], mybir.dt.bfloat16, tag="junk",
                                bufs=1, name=f"xsq{j}")
                sc = partials.tile([128, 2], F32, tag="sc", bufs=2,
                                   name=f"sc{j}")
                ssq, rms = sc[:, 0:1], sc[:, 1:2]
                nc.scalar.activation(xsq, xf,
                                     mybir.ActivationFunctionType.Square,
                                     accum_out=ssq)
                nc.scalar.activation(rms, ssq,
                                     mybir.ActivationFunctionType.Sqrt,
                                     bias=eps_t, scale=1.0 / k)
                nc.vector.reciprocal(invblk[:, j:j + 1], rms)
                x16 = f16s.tile([128, k], F16, tag="x16", bufs=2,
                                name=f"x16_{j}")
                nc.vector.tensor_tensor(x16, xf, gamma_rep,
                                        mybir.AluOpType.mult)
                xT = f16s.tile([128, kt, 128], F16, tag="xT",
                               bufs=strip_blk + 1, name=f"xT{j}")
                nc.sync.dma_start_transpose(out=xT, in_=x16)
                xT_tiles[j] = xT

            def finish_block(b):
                # inv*s = inv*t*2 for the block's strips in one DVE op
                lo = b * strip_blk
                nc.vector.tensor_scalar(
                    invsblk[:, lo:lo + strip_blk], invblk[:, lo:lo + strip_blk],
                    t_bc, 2.0, mybir.AluOpType.mult, mybir.AluOpType.mult)

            # prep the first block of strips during the collective wait so
            # DVE/ACT fill the window and the PE has xT ready when qT lands
            for j in range(strip_blk):
                prep_strip(j)
            finish_block(0)

            # ---- phase W2: quantize ----------------------------------------
            # nm = (w < -t);  q = (w > t) - nm   (2 DVE ops, fp16 outs)
            # qQ[q][p, u, o] = q(w[o, (4q+u)*128+p]); k-quarter tensors.
            n_kq = max(1, kt // 4)
            kq = kt // n_kq
            qQs = [qtp.tile([128, kq, dout_sh], F16, tag=f"qQ{q}",
                            name=f"qQ{q}") for q in range(n_kq)]
            for i in range(kt):
                if i < res_w:
                    wtT = wtiles[i]
                else:
                    # re-streams alternate scalar/sync queues (gpsimd stays
                    # clean for the collective) to double prefetch rate
                    wtT = f32s.tile([128, dout_sh], F32, tag="wstage", bufs=3,
                                    name=f"wq{i}")
                    reng = nc.scalar if i % 2 == 0 else nc.sync
                    reng.dma_start(out=wtT,
                                   in_=w_d[i * 128:(i + 1) * 128, :])
                # u = Sign(w + t) on ACT (exact near the boundary by
                # Sterbenz); nm = (u < 0) == (w < -t); q = (w > t) - nm.
                # Splitting one compare to ACT leaves DVE ~1.6us/tile so the
                # PE (1.05us/tile) trails the quantize much less.
                u = f16s.tile([128, dout_sh], F16, tag="nm", bufs=2,
                              name=f"u{i}")
                nc.scalar.activation(u, wtT,
                                     mybir.ActivationFunctionType.Sign,
                                     bias=t_bc)
                nm = f16s.tile([128, dout_sh], F16, tag="nm01", bufs=2,
                               name=f"nm{i}")
                nc.vector.tensor_scalar(nm, u, 0.0, None,
                                        mybir.AluOpType.is_lt)
                nc.vector.scalar_tensor_tensor(
                    qQs[i // kq][:, i % kq, :], wtT, t_bc, nm,
                    mybir.AluOpType.is_gt, mybir.AluOpType.subtract)

            # ---- blocked main loop: j-outer / t-mid / d-inner --------------
            for b in range(n_blk):
                for j in range(b * strip_blk, (b + 1) * strip_blk):
                    xT = xT_tiles[j]
                    pss = [psum.tile([128, 512], F32, tag="mm",
                                     name=f"ps{j}_{d}") for d in range(4)]
                    last = (j == n_strip - 1)

                    def epi(j, d, ps):
                        ob = outp.tile([128, 512], F32, tag="ob", bufs=3,
                                       name=f"ob{j}_{d}")
                        # alternate ACT/DVE so PSUM drain never queues
                        # behind the quantize Sign burst on the ACT FIFO
                        if d % 2 == 0:
                            nc.scalar.activation(
                                out=ob, in_=ps,
                                func=mybir.ActivationFunctionType.Copy,
                                scale=invsblk[:, j:j + 1])
                        else:
                            nc.vector.tensor_scalar(
                                ob, ps, invsblk[:, j:j + 1], None,
                                mybir.AluOpType.mult)
                        # alternate out-DMA queues to halve drain backlog
                        oeng = nc.gpsimd if d % 2 == 0 else nc.sync
                        oeng.dma_start(
                            out=out_d[j * 128:(j + 1) * 128,
                                      d * 512:(d + 1) * 512],
                            in_=ob)

                    if last:
                        # d-outer for the final strip: each psum group
                        # finishes 4x earlier so the tail epilogue + out-DMA
                        # overlaps the remaining matmuls
                        for d in range(4):
                            for t in range(kt):
                                nc.tensor.matmul(
                                    pss[d], lhsT=xT[:, t, :],
                                    rhs=qQs[t // kq][:, t % kq,
                                                     d * 512:(d + 1) * 512],
                                    start=(t == 0), stop=(t == kt - 1))
                            epi(j, d, pss[d])
                    else:
                        for t in range(kt):
                            for d in range(4):
                                nc.tensor.matmul(
                                    pss[d], lhsT=xT[:, t, :],
                                    rhs=qQs[t // kq][:, t % kq,
                                                     d * 512:(d + 1) * 512],
                                    start=(t == 0), stop=(t == kt - 1))
                        for d in range(4):
                            epi(j, d, pss[d])
                # prefetch-prep the next block while this one multiplies
                if b + 1 < n_blk:
                    for j in range((b + 1) * strip_blk, (b + 2) * strip_blk):
                        prep_strip(j)
                    finish_block(b + 1)

    nc.compile()
    return nc


_NC_CACHE = {}


def _get_nc():
    if "nc" not in _NC_CACHE:
        _NC_CACHE["nc"] = build_nc()
    return _NC_CACHE["nc"]


def kernel(x, weight, gamma):
    x = np.ascontiguousarray(np.asarray(x, dtype=np.float32))
    weight = np.ascontiguousarray(np.asarray(weight, dtype=np.float32))
    gamma = np.ascontiguousarray(np.asarray(gamma, dtype=np.float32))

    xf = x.reshape(B * S, K)
    wT = np.ascontiguousarray(weight.T)
    in_maps = []
    for c in range(N_CORES):
        rg, cg = c // CG, c % CG
        in_maps.append({
            "x_sh": xf[rg * TOK_SH:(rg + 1) * TOK_SH],
            "w_shT": np.ascontiguousarray(wT[:, cg * DOUT_SH:(cg + 1) * DOUT_SH]),
            "gamma": gamma,
        })

    nc = _get_nc()
    res = run_bass_kernel_spmd(nc, in_maps, list(range(N_CORES))).results

    out = np.empty((B * S, DOUT), dtype=np.float32)
    for c in range(N_CORES):
        rg, cg = c // CG, c % CG
        out[rg * TOK_SH:(rg + 1) * TOK_SH,
            cg * DOUT_SH:(cg + 1) * DOUT_SH] = res[c]["out_sh"]
    return out.reshape(B, S, DOUT)
